# revision 8
# baseline (speedup 1.0000x reference)
"""Trainium2 Bass kernel for nn_JointPairHead: edge gather + LN + 3x(Linear->BN->ReLU) -> logits.

Sharding (per the data-parallel hint): shard edge_index and gathered edge
features across 8 cores; params replicated. The edge-feature gather
h = x[src] + x[dst] happens host-side (numpy fancy indexing) and each core
receives its contiguous [E/8, 256] bf16 slab pre-arranged in the on-device
edge-major tile layout. BN batch stats cross-core via AllReduce of per-shard
sum/sumsq.

Device dataflow (per core, E_shard = 32768 edges, 64 blocks of 512,
all-bf16 compute with fp32 stat accumulation):
  P0: stream h blocks -> LayerNorm (stats via accum_out, normalize via
      tensor_scalar) -> xbar DMA transpose to feature-major [256d, 512e]
      -> matmul z0 = W0f @ hnT (ln_w folded into W0) -> z0 into SBUF-resident
      z-store (bf16) with per-channel sum/sumsq accumulation.
  AllReduce stats -> BN affine a,b
  P1/P2: relu(a*z+b) from z-store (feature-major already) -> matmul -> z-store
  P3: relu-affine -> matmul w_out -> logits -> DRAM
No DRAM spills: activations stay SBUF-resident for the whole net.
"""

import numpy as np

N_NODES = 50000
D = 256
E_TOT = 262144
NCORES = 8
ESH = E_TOT // NCORES          # 32768 edges per core
EBLK = 512                     # edges per block
NB = ESH // EBLK               # 64 blocks
EROW = ESH // 128              # 256 gather rows per partition
CH = 4096                      # edges per streamed chunk (8 blocks)
EPS = 1e-5
NL = 3


def build_nc(num_devices=NCORES):
    import concourse.mybir as mybir
    import concourse.tile as tile
    from concourse import bacc

    f32 = mybir.dt.float32
    bf16 = mybir.dt.bfloat16
    A = mybir.ActivationFunctionType
    ALU = mybir.AluOpType
    AX = mybir.AxisListType

    groups_dev = [list(range(num_devices))]
    inv_d = 1.0 / D
    inv_e = 1.0 / E_TOT

    nc = bacc.Bacc("TRN2", target_bir_lowering=False, debug=False,
                   num_devices=num_devices)

    # ---- kernel I/O ----
    hin = nc.dram_tensor("hin", [128, EROW, D], bf16, kind="ExternalInput").ap()
    wts = [nc.dram_tensor(f"w{i}t", [D, D], bf16, kind="ExternalInput").ap()
           for i in range(NL)]
    wot = nc.dram_tensor("wot", [D, 1], bf16, kind="ExternalInput").ap()
    gam = nc.dram_tensor("gam", [NL, D], f32, kind="ExternalInput").ap()
    bet = nc.dram_tensor("bet", [NL, D], f32, kind="ExternalInput").ap()
    out = nc.dram_tensor("out", [ESH], f32, kind="ExternalOutput").ap()

    ccin = [nc.dram_tensor(f"ccin{i}", [128, 4], f32, kind="Internal").ap()
            for i in range(NL)]
    cc_space = "Shared" if num_devices > 4 else "Local"
    ccout = [nc.dram_tensor(f"ccout{i}", [128, 4], f32, kind="Internal",
                            addr_space=cc_space).ap()
             for i in range(NL)]

    with tile.TileContext(nc) as tc:
        with (
            tc.tile_pool(name="const", bufs=1) as cpool,
            tc.tile_pool(name="io", bufs=2) as iop,
            tc.tile_pool(name="work", bufs=2) as wp,
            tc.tile_pool(name="stats", bufs=1) as sp,
            tc.tile_pool(name="psum", bufs=4, space="PSUM") as pp,
            tc.tile_pool(name="psum2", bufs=2, space="PSUM") as pp2,
        ):
            # ---- persistent SBUF state ----
            zst = cpool.tile([128, 2, ESH], bf16, name="zst")   # z store
            wsb = []
            for i in range(NL):
                chunks = []
                for c in range(2):
                    t = cpool.tile([128, D], bf16, name=f"w{i}c{c}")
                    nc.sync.dma_start(out=t[:], in_=wts[i][c * 128:(c + 1) * 128, :])
                    chunks.append(t)
                wsb.append(chunks)
            wot_sb = []
            for c in range(2):
                t = cpool.tile([128, 1], bf16, name=f"wo{c}")
                nc.sync.dma_start(out=t[:], in_=wot[c * 128:(c + 1) * 128, :])
                wot_sb.append(t)
            gam_sb, bet_sb = [], []
            for i in range(NL):
                g = cpool.tile([128, 2], f32, name=f"gam{i}")
                b = cpool.tile([128, 2], f32, name=f"bet{i}")
                for c in range(2):
                    nc.sync.dma_start(out=g[:, c:c + 1],
                                      in_=gam[i, c * 128:(c + 1) * 128])
                    nc.sync.dma_start(out=b[:, c:c + 1],
                                      in_=bet[i, c * 128:(c + 1) * 128])
                gam_sb.append(g)
                bet_sb.append(b)

            Sz = [[sp.tile([128, NB], f32, name=f"Sz{i}_{c}") for c in range(2)]
                  for i in range(NL)]
            SSz = [[sp.tile([128, NB], f32, name=f"SSz{i}_{c}") for c in range(2)]
                   for i in range(NL)]
            a_ab = [sp.tile([128, 2], f32, name=f"a{i}") for i in range(NL)]
            b_ab = [sp.tile([128, 2], f32, name=f"b{i}") for i in range(NL)]

            def produce_z(li, blk, rhs_of_k):
                """matmul z_li over k-chunks -> z-store + stat accumulation."""
                for c in range(2):
                    zps = pp.tile([128, EBLK], f32, name="zps", tag="zps")
                    for k in range(2):
                        nc.tensor.matmul(
                            out=zps[:],
                            lhsT=wsb[li][k][:, c * 128:(c + 1) * 128],
                            rhs=rhs_of_k(k),
                            start=(k == 0), stop=(k == 1))
                    zcol = zst[:, c, blk * EBLK:(blk + 1) * EBLK]
                    nc.scalar.activation(out=zcol, in_=zps[:], func=A.Copy,
                                         accum_out=Sz[li][c][:, blk:blk + 1])
                    zqs = wp.tile([128, EBLK], bf16, name="zqs", tag="zqs")
                    nc.vector.scalar_tensor_tensor(
                        out=zqs[:], in0=zcol, scalar=1.0, in1=zcol,
                        op0=ALU.mult, op1=ALU.mult,
                        accum_out=SSz[li][c][:, blk:blk + 1])

            # ================= Phase 0: LN + layer 0 =================
            for ci in range(ESH // CH):
                hch = iop.tile([128, CH // 128, D], bf16, name="hch", tag="hch")
                nc.sync.dma_start(
                    out=hch[:],
                    in_=hin[:, ci * (CH // 128):(ci + 1) * (CH // 128), :])
                for b in range(CH // EBLK):
                    blk = ci * (CH // EBLK) + b
                    h4 = hch[:, 4 * b:4 * b + 4, :]
                    hn = wp.tile([128, 4, D], bf16, name="hn", tag="hn")
                    Sln = wp.tile([128, 4], f32, name="Sln", tag="Sln")
                    SSln = wp.tile([128, 4], f32, name="SSln", tag="SSln")
                    mu = wp.tile([128, 4], f32, name="mu", tag="mu")
                    mu2 = wp.tile([128, 4], f32, name="mu2", tag="mu2")
                    var = wp.tile([128, 4], f32, name="var", tag="var")
                    inv = wp.tile([128, 4], f32, name="inv", tag="inv")
                    rs = wp.tile([128, 4], f32, name="rs", tag="rs")
                    bneg = wp.tile([128, 4], f32, name="bneg", tag="bneg")
                    for j in range(4):
                        # copy pass exists only for the per-group row sums;
                        # hn is scratch here (overwritten by normalize below)
                        nc.vector.tensor_scalar(
                            out=hn[:, j, :], in0=h4[:, j, :], scalar1=1.0,
                            scalar2=0.0, op0=ALU.mult, op1=ALU.add,
                            accum_out=Sln[:, j:j + 1])
                    for j in range(4):
                        nc.vector.scalar_tensor_tensor(
                            out=hn[:, j, :], in0=h4[:, j, :], scalar=1.0,
                            in1=h4[:, j, :], op0=ALU.mult, op1=ALU.mult,
                            accum_out=SSln[:, j:j + 1])
                    nc.scalar.activation(out=mu[:], in_=Sln[:], func=A.Copy,
                                         scale=inv_d)
                    nc.vector.scalar_tensor_tensor(
                        out=mu2[:], in0=mu[:], scalar=1.0, in1=mu[:],
                        op0=ALU.mult, op1=ALU.mult)
                    nc.vector.scalar_tensor_tensor(
                        out=var[:], in0=SSln[:], scalar=inv_d, in1=mu2[:],
                        op0=ALU.mult, op1=ALU.subtract)
                    nc.vector.tensor_scalar_add(out=var[:], in0=var[:],
                                                scalar1=EPS)
                    nc.vector.reciprocal(out=inv[:], in_=var[:])
                    nc.scalar.sqrt(out=rs[:], in_=inv[:])
                    nc.vector.scalar_tensor_tensor(
                        out=bneg[:], in0=mu[:], scalar=-1.0, in1=rs[:],
                        op0=ALU.mult, op1=ALU.mult)
                    for j in range(4):
                        nc.vector.tensor_scalar(
                            out=hn[:, j, :], in0=h4[:, j, :],
                            scalar1=rs[:, j:j + 1], scalar2=bneg[:, j:j + 1],
                            op0=ALU.mult, op1=ALU.add)
                    # xbar transpose: [128e, (g,c,p)] -> [128p, g, c, 128e]
                    hT = wp.tile([128, 4, 2, 128], bf16, name="hT", tag="hT")
                    nc.sync.dma_start_transpose(
                        out=hT[:], in_=hn[:].rearrange("p a d -> p (a d)"))
                    produce_z(0, blk, lambda k: hT[:, :, k, :])

            # ============ stats AllReduce + BN affine ============
            def finalize_stats(li):
                st4 = sp.tile([128, 4], f32, name=f"st4_{li}")
                for c in range(2):
                    nc.vector.reduce_sum(out=st4[:, c:c + 1], in_=Sz[li][c][:],
                                         axis=AX.X)
                    nc.vector.reduce_sum(out=st4[:, 2 + c:3 + c],
                                         in_=SSz[li][c][:], axis=AX.X)
                nc.sync.dma_start(out=ccin[li][:, :], in_=st4[:])
                if num_devices == 1:
                    nc.sync.dma_start(out=ccout[li][:, :], in_=ccin[li][:, :])
                else:
                    nc.gpsimd.collective_compute(
                        "AllReduce", ALU.add, replica_groups=groups_dev,
                        ins=[ccin[li][:, :]], outs=[ccout[li][:, :]])
                gst = sp.tile([128, 4], f32, name=f"gst{li}")
                nc.sync.dma_start(out=gst[:], in_=ccout[li][:, :])
                bmu = sp.tile([128, 2], f32, name=f"bmu{li}")
                bmu2 = sp.tile([128, 2], f32, name=f"bmu2{li}")
                bvar = sp.tile([128, 2], f32, name=f"bvar{li}")
                binv = sp.tile([128, 2], f32, name=f"binv{li}")
                brs = sp.tile([128, 2], f32, name=f"brs{li}")
                tt = sp.tile([128, 2], f32, name=f"tt{li}")
                nc.scalar.mul(out=bmu[:], in_=gst[:, 0:2], mul=inv_e)
                nc.scalar.square(out=bmu2[:], in_=bmu[:])
                nc.vector.scalar_tensor_tensor(
                    out=bvar[:], in0=gst[:, 2:4], scalar=inv_e, in1=bmu2[:],
                    op0=ALU.mult, op1=ALU.subtract)
                nc.vector.tensor_scalar_add(out=bvar[:], in0=bvar[:], scalar1=EPS)
                nc.vector.reciprocal(out=binv[:], in_=bvar[:])
                nc.scalar.sqrt(out=brs[:], in_=binv[:])
                nc.vector.tensor_mul(out=a_ab[li][:], in0=gam_sb[li][:], in1=brs[:])
                nc.vector.tensor_mul(out=tt[:], in0=a_ab[li][:], in1=bmu[:])
                nc.vector.tensor_sub(out=b_ab[li][:], in0=bet_sb[li][:], in1=tt[:])

            finalize_stats(0)

            # ================= Phases 1..2 =================
            def relu_affine(li, blk, c):
                """hn_c = relu(a*z + b) from z-store, bf16, feature-major."""
                zcol = zst[:, c, blk * EBLK:(blk + 1) * EBLK]
                t = wp.tile([128, EBLK], bf16, name=f"aff{c}", tag=f"aff{c}")
                hn_c = wp.tile([128, EBLK], bf16, name=f"rhc{c}", tag=f"rhc{c}")
                nc.vector.tensor_scalar(
                    out=t[:], in0=zcol, scalar1=a_ab[li - 1][:, c:c + 1],
                    scalar2=b_ab[li - 1][:, c:c + 1], op0=ALU.mult, op1=ALU.add)
                nc.vector.tensor_scalar(
                    out=hn_c[:], in0=t[:], scalar1=0.0, scalar2=None,
                    op0=ALU.max)
                return hn_c

            for li in range(1, NL):
                for blk in range(NB):
                    hns = [relu_affine(li, blk, c) for c in range(2)]
                    produce_z(li, blk, lambda k: hns[k][:])
                finalize_stats(li)

            # ================= Phase 3: final projection =================
            for blk in range(NB):
                hns = [relu_affine(NL, blk, c) for c in range(2)]
                lps = pp2.tile([1, EBLK], f32, name="lps", tag="lps")
                for c in range(2):
                    nc.tensor.matmul(out=lps[:], lhsT=wot_sb[c][:], rhs=hns[c][:],
                                     start=(c == 0), stop=(c == 1))
                lsb = wp.tile([1, EBLK], f32, name="lsb", tag="lsb")
                nc.scalar.copy(out=lsb[:], in_=lps[:])
                nc.sync.dma_start(out=out[blk * EBLK:(blk + 1) * EBLK], in_=lsb[:])

    nc.compile()
    return nc


_NC = None


def _to_bf16(a):
    import ml_dtypes
    return np.asarray(a, dtype=np.float32).astype(ml_dtypes.bfloat16)


def kernel(**inputs):
    global _NC

    x = np.asarray(inputs["x"], dtype=np.float32)
    ei = np.asarray(inputs["jg_edge_index"]).astype(np.int64)
    ln_w = np.asarray(inputs["ln_w"], dtype=np.float32)
    Ws = np.asarray(inputs["Ws"], dtype=np.float32)
    gammas = np.asarray(inputs["gammas"], dtype=np.float32)
    betas = np.asarray(inputs["betas"], dtype=np.float32)
    W_out = np.asarray(inputs["W_out"], dtype=np.float32)

    # fold ln_w into layer-0 weight; lhsT layout = W.T ([in,out])
    W0f = Ws[0] * ln_w[None, :]
    wts = [_to_bf16(W0f.T), _to_bf16(Ws[1].T), _to_bf16(Ws[2].T)]
    wot = _to_bf16(W_out.reshape(1, D).T)

    # host-side gather of the edge features (data-parallel sharding of
    # "gathered edge features" per the sharding strategy)
    h = x[ei[0], :] + x[ei[1], :]                    # [E, 256] fp32
    h_bf = _to_bf16(h)

    if _NC is None:
        _NC = build_nc()

    in_maps = []
    for c in range(NCORES):
        hc = h_bf[c * ESH:(c + 1) * ESH]
        # edge e -> (partition e%128, row e//128) device layout
        hdev = np.ascontiguousarray(
            hc.reshape(EROW, 128, D).transpose(1, 0, 2))
        in_maps.append({
            "hin": hdev,
            "w0t": wts[0], "w1t": wts[1], "w2t": wts[2],
            "wot": wot,
            "gam": gammas,
            "bet": betas,
        })

    global _last_in_maps
    _last_in_maps = in_maps

    from concourse import bass_utils
    res = bass_utils.run_bass_kernel_spmd(_NC, in_maps, core_ids=list(range(NCORES)))
    return np.concatenate([np.asarray(res.results[c]["out"], dtype=np.float32)
                           for c in range(NCORES)], axis=0)


_last_in_maps = None


# revision 12
# speedup vs baseline: 1.2555x; 1.2555x over previous
"""Trainium2 Bass kernel for nn_JointPairHead: edge gather + LN + 3x(Linear->BN->ReLU) -> logits.

Sharding (per the data-parallel hint): shard edge_index and gathered edge
features across 8 cores; params replicated. The edge-feature gather
h = x[src] + x[dst] happens host-side (numpy fancy indexing) and each core
receives its contiguous [E/8, 256] bf16 slab pre-arranged in the on-device
edge-major tile layout. BN batch stats cross-core via AllReduce of per-shard
sum/sumsq.

Device dataflow (per core, E_shard = 32768 edges, 64 blocks of 512,
all-bf16 compute with fp32 stat accumulation, SBUF-resident activations):
  P0: stream h blocks -> LN stats (sum via tensor_scalar accum, sumsq via
      scalar_tensor_tensor accum; scalar math batched per 8-block chunk)
      -> normalize via tensor_scalar -> xbar DMA transpose to feature-major
      -> matmul z0 = W0f @ hnT (ln_w folded into W0) -> z0 into z-store with
      per-channel sum (ACT copy accum) / sumsq (ACT Square accum from PSUM).
  AllReduce stats -> BN affine.
  P1/P2: when all gammas>0, relu(a*z+b) = a*max(z,t)+b with t = bmu-beta/a;
      the per-channel +b term is folded into the next matmul as a bias
      (c = W@b) applied in the PSUM->z-store copy. One DVE op per chunk.
  P3: fused relu-affine -> matmul w_out (+wot@b bias) -> logits -> DRAM.
"""

import numpy as np

N_NODES = 50000
D = 256
E_TOT = 262144
NCORES = 8
ESH = E_TOT // NCORES          # 32768 edges per core
EBLK = 512                     # edges per block
NB = ESH // EBLK               # 64 blocks
EROW = ESH // 128              # 256 gather rows per partition
CH = 4096                      # edges per streamed chunk (8 blocks)
CB = CH // EBLK                # blocks per chunk
EPS = 1e-5
NL = 3


def build_nc(num_devices=NCORES, fused=True):
    import concourse.mybir as mybir
    import concourse.tile as tile
    from concourse import bacc

    f32 = mybir.dt.float32
    bf16 = mybir.dt.bfloat16
    A = mybir.ActivationFunctionType
    ALU = mybir.AluOpType
    AX = mybir.AxisListType

    groups_dev = [list(range(num_devices))]
    inv_d = 1.0 / D
    inv_e = 1.0 / E_TOT

    nc = bacc.Bacc("TRN2", target_bir_lowering=False, debug=False,
                   num_devices=num_devices)

    # ---- kernel I/O ----
    hin = nc.dram_tensor("hin", [128, EROW, D], bf16, kind="ExternalInput").ap()
    wts = [nc.dram_tensor(f"w{i}t", [D, D], bf16, kind="ExternalInput").ap()
           for i in range(NL)]
    wot = nc.dram_tensor("wot", [D, 1], bf16, kind="ExternalInput").ap()
    gam = nc.dram_tensor("gam", [NL, D], f32, kind="ExternalInput").ap()
    bet = nc.dram_tensor("bet", [NL, D], f32, kind="ExternalInput").ap()
    out = nc.dram_tensor("out", [ESH], f32, kind="ExternalOutput").ap()

    ccin = [nc.dram_tensor(f"ccin{i}", [128, 4], f32, kind="Internal").ap()
            for i in range(NL)]
    cc_space = "Shared" if num_devices > 4 else "Local"
    ccout = [nc.dram_tensor(f"ccout{i}", [128, 4], f32, kind="Internal",
                            addr_space=cc_space).ap()
             for i in range(NL)]

    with tile.TileContext(nc) as tc:
        with (
            tc.tile_pool(name="const", bufs=1) as cpool,
            tc.tile_pool(name="io", bufs=2) as iop,
            tc.tile_pool(name="work", bufs=3) as wp,
            tc.tile_pool(name="stats", bufs=1) as sp,
            tc.tile_pool(name="psum", bufs=4, space="PSUM") as pp,
            tc.tile_pool(name="psum2", bufs=2, space="PSUM") as pp2,
            tc.tile_pool(name="psum3", bufs=1, space="PSUM") as pp3,
        ):
            # ---- persistent SBUF state ----
            zst = cpool.tile([128, 2, ESH], bf16, name="zst")   # z store
            wsb = []
            for i in range(NL):
                chunks = []
                for c in range(2):
                    t = cpool.tile([128, D], bf16, name=f"w{i}c{c}")
                    nc.sync.dma_start(out=t[:], in_=wts[i][c * 128:(c + 1) * 128, :])
                    chunks.append(t)
                wsb.append(chunks)
            wot_sb = []
            for c in range(2):
                t = cpool.tile([128, 1], bf16, name=f"wo{c}")
                nc.sync.dma_start(out=t[:], in_=wot[c * 128:(c + 1) * 128, :])
                wot_sb.append(t)
            gam_sb, bet_sb = [], []
            for i in range(NL):
                g = cpool.tile([128, 2], f32, name=f"gam{i}")
                b = cpool.tile([128, 2], f32, name=f"bet{i}")
                for c in range(2):
                    nc.sync.dma_start(out=g[:, c:c + 1],
                                      in_=gam[i, c * 128:(c + 1) * 128])
                    nc.sync.dma_start(out=b[:, c:c + 1],
                                      in_=bet[i, c * 128:(c + 1) * 128])
                gam_sb.append(g)
                bet_sb.append(b)

            Sz = [[sp.tile([128, NB], f32, name=f"Sz{i}_{c}") for c in range(2)]
                  for i in range(NL)]
            SSz = [[sp.tile([128, NB], f32, name=f"SSz{i}_{c}") for c in range(2)]
                   for i in range(NL)]
            a_ab = [sp.tile([128, 2], f32, name=f"a{i}") for i in range(NL)]
            b_ab = [sp.tile([128, 2], f32, name=f"b{i}") for i in range(NL)]
            t_ab = [sp.tile([128, 2], f32, name=f"t{i}") for i in range(NL)]
            # +W_{li}@b_{li-1} bias for the PSUM->z-store copy (fused path)
            cbias = [sp.tile([128, 2], f32, name=f"cb{i}") for i in range(NL)]
            obias = sp.tile([1, 1], f32, name="obias")

            def produce_z(li, blk, rhs_of_k, sq_on_act):
                """matmul z_li over k-chunks -> z-store + stat accumulation."""
                use_bias = fused and li > 0
                for c in range(2):
                    zps = pp.tile([128, EBLK], f32, name="zps", tag="zps")
                    for k in range(2):
                        nc.tensor.matmul(
                            out=zps[:],
                            lhsT=wsb[li][k][:, c * 128:(c + 1) * 128],
                            rhs=rhs_of_k(k),
                            start=(k == 0), stop=(k == 1))
                    zcol = zst[:, c, blk * EBLK:(blk + 1) * EBLK]
                    if use_bias:
                        nc.scalar.activation(
                            out=zcol, in_=zps[:], func=A.Identity,
                            bias=cbias[li][:, c:c + 1],
                            accum_out=Sz[li][c][:, blk:blk + 1])
                    else:
                        nc.scalar.activation(
                            out=zcol, in_=zps[:], func=A.Copy,
                            accum_out=Sz[li][c][:, blk:blk + 1])
                    zqs = wp.tile([128, EBLK], bf16, name="zqs", tag="zqs")
                    if sq_on_act:
                        # note: squares of pre-bias PSUM would be wrong when
                        # use_bias; read the biased z-store column instead
                        src = zcol if use_bias else zps[:]
                        nc.scalar.activation(
                            out=zqs[:], in_=src, func=A.Square,
                            accum_out=SSz[li][c][:, blk:blk + 1])
                    else:
                        nc.vector.scalar_tensor_tensor(
                            out=zqs[:], in0=zcol, scalar=1.0, in1=zcol,
                            op0=ALU.mult, op1=ALU.mult,
                            accum_out=SSz[li][c][:, blk:blk + 1])

            # ================= Phase 0: LN + layer 0 =================
            for ci in range(ESH // CH):
                hch = iop.tile([128, CB * 4, D], bf16, name="hch", tag="hch")
                nc.sync.dma_start(
                    out=hch[:],
                    in_=hin[:, ci * (CB * 4):(ci + 1) * (CB * 4), :])
                Sln = wp.tile([128, CB * 4], f32, name="Sln", tag="Sln")
                SSln = wp.tile([128, CB * 4], f32, name="SSln", tag="SSln")
                for b in range(CB):
                    for j in range(4):
                        g = 4 * b + j
                        scr = wp.tile([128, D], bf16, name="scr", tag="scr")
                        nc.vector.tensor_scalar(
                            out=scr[:], in0=hch[:, g, :], scalar1=1.0,
                            scalar2=0.0, op0=ALU.mult, op1=ALU.add,
                            accum_out=Sln[:, g:g + 1])
                        nc.vector.scalar_tensor_tensor(
                            out=scr[:], in0=hch[:, g, :], scalar=1.0,
                            in1=hch[:, g, :], op0=ALU.mult, op1=ALU.mult,
                            accum_out=SSln[:, g:g + 1])
                # batched LN scalar math for the whole chunk [128, 32]
                mu = wp.tile([128, CB * 4], f32, name="mu", tag="mu")
                mu2 = wp.tile([128, CB * 4], f32, name="mu2", tag="mu2")
                var = wp.tile([128, CB * 4], f32, name="var", tag="var")
                inv = wp.tile([128, CB * 4], f32, name="inv", tag="inv")
                rs = wp.tile([128, CB * 4], f32, name="rs", tag="rs")
                bneg = wp.tile([128, CB * 4], f32, name="bneg", tag="bneg")
                nc.vector.tensor_scalar(
                    out=mu[:], in0=Sln[:], scalar1=inv_d, scalar2=None,
                    op0=ALU.mult)
                nc.vector.scalar_tensor_tensor(
                    out=mu2[:], in0=mu[:], scalar=1.0, in1=mu[:],
                    op0=ALU.mult, op1=ALU.mult)
                nc.vector.scalar_tensor_tensor(
                    out=var[:], in0=SSln[:], scalar=inv_d, in1=mu2[:],
                    op0=ALU.mult, op1=ALU.subtract)
                nc.vector.tensor_scalar_add(out=var[:], in0=var[:], scalar1=EPS)
                nc.vector.reciprocal(out=inv[:], in_=var[:])
                nc.scalar.sqrt(out=rs[:], in_=inv[:])
                nc.vector.scalar_tensor_tensor(
                    out=bneg[:], in0=mu[:], scalar=-1.0, in1=rs[:],
                    op0=ALU.mult, op1=ALU.mult)
                for b in range(CB):
                    blk = ci * CB + b
                    hn = wp.tile([128, 4, D], bf16, name="hn", tag="hn")
                    for j in range(4):
                        g = 4 * b + j
                        nc.vector.tensor_scalar(
                            out=hn[:, j, :], in0=hch[:, g, :],
                            scalar1=rs[:, g:g + 1], scalar2=bneg[:, g:g + 1],
                            op0=ALU.mult, op1=ALU.add)
                    # xbar transpose: [128e, (g,c,p)] -> [128p, g, c, 128e]
                    hT = wp.tile([128, 4, 2, 128], bf16, name="hT", tag="hT")
                    nc.sync.dma_start_transpose(
                        out=hT[:], in_=hn[:].rearrange("p a d -> p (a d)"))
                    produce_z(0, blk, lambda k: hT[:, :, k, :], sq_on_act=True)

            # ============ stats AllReduce + BN affine ============
            def finalize_stats(li):
                st4 = sp.tile([128, 4], f32, name=f"st4_{li}")
                for c in range(2):
                    nc.vector.reduce_sum(out=st4[:, c:c + 1], in_=Sz[li][c][:],
                                         axis=AX.X)
                    nc.vector.reduce_sum(out=st4[:, 2 + c:3 + c],
                                         in_=SSz[li][c][:], axis=AX.X)
                nc.sync.dma_start(out=ccin[li][:, :], in_=st4[:])
                if num_devices == 1:
                    nc.sync.dma_start(out=ccout[li][:, :], in_=ccin[li][:, :])
                else:
                    nc.gpsimd.collective_compute(
                        "AllReduce", ALU.add, replica_groups=groups_dev,
                        ins=[ccin[li][:, :]], outs=[ccout[li][:, :]])
                gst = sp.tile([128, 4], f32, name=f"gst{li}")
                nc.sync.dma_start(out=gst[:], in_=ccout[li][:, :])
                bmu = sp.tile([128, 2], f32, name=f"bmu{li}")
                bmu2 = sp.tile([128, 2], f32, name=f"bmu2{li}")
                bvar = sp.tile([128, 2], f32, name=f"bvar{li}")
                binv = sp.tile([128, 2], f32, name=f"binv{li}")
                brs = sp.tile([128, 2], f32, name=f"brs{li}")
                tt = sp.tile([128, 2], f32, name=f"tt{li}")
                nc.scalar.mul(out=bmu[:], in_=gst[:, 0:2], mul=inv_e)
                nc.scalar.square(out=bmu2[:], in_=bmu[:])
                nc.vector.scalar_tensor_tensor(
                    out=bvar[:], in0=gst[:, 2:4], scalar=inv_e, in1=bmu2[:],
                    op0=ALU.mult, op1=ALU.subtract)
                nc.vector.tensor_scalar_add(out=bvar[:], in0=bvar[:], scalar1=EPS)
                nc.vector.reciprocal(out=binv[:], in_=bvar[:])
                nc.scalar.sqrt(out=brs[:], in_=binv[:])
                nc.vector.tensor_mul(out=a_ab[li][:], in0=gam_sb[li][:], in1=brs[:])
                nc.vector.tensor_mul(out=tt[:], in0=a_ab[li][:], in1=bmu[:])
                nc.vector.tensor_sub(out=b_ab[li][:], in0=bet_sb[li][:], in1=tt[:])
                if fused:
                    # t = bmu - beta/a ;  c_{li+1} = W_{li+1} @ b ; obias = wot@b
                    ainv = sp.tile([128, 2], f32, name=f"ainv{li}")
                    boa = sp.tile([128, 2], f32, name=f"boa{li}")
                    nc.vector.reciprocal(out=ainv[:], in_=a_ab[li][:])
                    nc.vector.tensor_mul(out=boa[:], in0=bet_sb[li][:],
                                         in1=ainv[:])
                    nc.vector.tensor_sub(out=t_ab[li][:], in0=bmu[:], in1=boa[:])
                    bb = sp.tile([128, 2], bf16, name=f"bb{li}")
                    nc.scalar.copy(out=bb[:], in_=b_ab[li][:])
                    if li + 1 < NL:
                        cps = pp3.tile([128, 2], f32, name="cps", tag="cps")
                        for c in range(2):
                            for k in range(2):
                                nc.tensor.matmul(
                                    out=cps[:, c:c + 1],
                                    lhsT=wsb[li + 1][k][:, c * 128:(c + 1) * 128],
                                    rhs=bb[:, k:k + 1],
                                    start=(k == 0), stop=(k == 1))
                        nc.scalar.copy(out=cbias[li + 1][:], in_=cps[:])
                    else:
                        ops = pp3.tile([1, 1], f32, name="ops", tag="ops")
                        for k in range(2):
                            nc.tensor.matmul(out=ops[:], lhsT=wot_sb[k][:],
                                             rhs=bb[:, k:k + 1],
                                             start=(k == 0), stop=(k == 1))
                        nc.scalar.copy(out=obias[:], in_=ops[:])

            finalize_stats(0)

            # ================= Phases 1..2 =================
            def relu_affine(li, blk, c):
                """hn_c = relu(a*z + b) from z-store, bf16, feature-major.
                Fused: a*max(z,t); the +b is folded into the next bias."""
                zcol = zst[:, c, blk * EBLK:(blk + 1) * EBLK]
                hn_c = wp.tile([128, EBLK], bf16, name=f"rhc{c}", tag=f"rhc{c}")
                if fused:
                    nc.vector.tensor_scalar(
                        out=hn_c[:], in0=zcol,
                        scalar1=t_ab[li - 1][:, c:c + 1],
                        scalar2=a_ab[li - 1][:, c:c + 1],
                        op0=ALU.max, op1=ALU.mult)
                else:
                    t = wp.tile([128, EBLK], bf16, name=f"aff{c}", tag=f"aff{c}")
                    nc.vector.tensor_scalar(
                        out=t[:], in0=zcol, scalar1=a_ab[li - 1][:, c:c + 1],
                        scalar2=b_ab[li - 1][:, c:c + 1], op0=ALU.mult,
                        op1=ALU.add)
                    nc.vector.tensor_scalar(
                        out=hn_c[:], in0=t[:], scalar1=0.0, scalar2=None,
                        op0=ALU.max)
                return hn_c

            for li in range(1, NL):
                for blk in range(NB):
                    hns = [relu_affine(li, blk, c) for c in range(2)]
                    produce_z(li, blk, lambda k: hns[k][:], sq_on_act=False)
                finalize_stats(li)

            # ================= Phase 3: final projection =================
            for blk in range(NB):
                hns = [relu_affine(NL, blk, c) for c in range(2)]
                lps = pp2.tile([1, EBLK], f32, name="lps", tag="lps")
                for c in range(2):
                    nc.tensor.matmul(out=lps[:], lhsT=wot_sb[c][:], rhs=hns[c][:],
                                     start=(c == 0), stop=(c == 1))
                lsb = wp.tile([1, EBLK], f32, name="lsb", tag="lsb")
                if fused:
                    nc.scalar.activation(out=lsb[:], in_=lps[:], func=A.Identity,
                                         bias=obias[:, 0:1])
                else:
                    nc.scalar.copy(out=lsb[:], in_=lps[:])
                nc.sync.dma_start(out=out[blk * EBLK:(blk + 1) * EBLK], in_=lsb[:])

    nc.compile()
    return nc


_NC = None
_NC_KEY = None


def _to_bf16(a):
    import ml_dtypes
    return np.asarray(a, dtype=np.float32).astype(ml_dtypes.bfloat16)


def kernel(**inputs):
    global _NC, _NC_KEY

    x = np.asarray(inputs["x"], dtype=np.float32)
    ei = np.asarray(inputs["jg_edge_index"]).astype(np.int64)
    ln_w = np.asarray(inputs["ln_w"], dtype=np.float32)
    Ws = np.asarray(inputs["Ws"], dtype=np.float32)
    gammas = np.asarray(inputs["gammas"], dtype=np.float32)
    betas = np.asarray(inputs["betas"], dtype=np.float32)
    W_out = np.asarray(inputs["W_out"], dtype=np.float32)

    # fold ln_w into layer-0 weight; lhsT layout = W.T ([in,out])
    W0f = Ws[0] * ln_w[None, :]
    wts = [_to_bf16(W0f.T), _to_bf16(Ws[1].T), _to_bf16(Ws[2].T)]
    wot = _to_bf16(W_out.reshape(1, D).T)

    # relu(a*z+b) = a*max(z, t)+b identity requires a = gamma*rsqrt(var) > 0
    fused = bool((gammas > 0).all())

    # host-side gather of the edge features (data-parallel sharding of
    # "gathered edge features" per the sharding strategy)
    h = x[ei[0], :] + x[ei[1], :]                    # [E, 256] fp32
    h_bf = _to_bf16(h)

    if _NC is None or _NC_KEY != fused:
        _NC = build_nc(fused=fused)
        _NC_KEY = fused

    in_maps = []
    for c in range(NCORES):
        hc = h_bf[c * ESH:(c + 1) * ESH]
        # edge e -> (partition e%128, row e//128) device layout
        hdev = np.ascontiguousarray(
            hc.reshape(EROW, 128, D).transpose(1, 0, 2))
        in_maps.append({
            "hin": hdev,
            "w0t": wts[0], "w1t": wts[1], "w2t": wts[2],
            "wot": wot,
            "gam": gammas,
            "bet": betas,
        })

    global _last_in_maps
    _last_in_maps = in_maps

    from concourse import bass_utils
    res = bass_utils.run_bass_kernel_spmd(_NC, in_maps, core_ids=list(range(NCORES)))
    return np.concatenate([np.asarray(res.results[c]["out"], dtype=np.float32)
                           for c in range(NCORES)], axis=0)


_last_in_maps = None


# revision 15
# speedup vs baseline: 1.3523x; 1.0771x over previous
"""Trainium2 Bass kernel for nn_JointPairHead: edge gather + LN + 3x(Linear->BN->ReLU) -> logits.

Sharding (per the data-parallel hint): shard edge_index and gathered edge
features across 8 cores; params replicated. The edge-feature gather
h = x[src] + x[dst] happens host-side (numpy fancy indexing) and each core
receives its contiguous [E/8, 256] bf16 slab pre-arranged in the on-device
edge-major tile layout. BN batch stats cross-core via AllReduce of per-shard
sum/sumsq.

Device dataflow (per core, E_shard = 32768 edges, 64 blocks of 512,
all-bf16 compute with fp32 stat accumulation, SBUF-resident activations):
  P0: stream h blocks -> LN stats (sum via tensor_scalar accum, sumsq via
      scalar_tensor_tensor accum; scalar math batched per 8-block chunk)
      -> normalize via tensor_scalar -> xbar DMA transpose to feature-major
      -> matmul z0 = W0f @ hnT (ln_w folded into W0) -> z0 into z-store with
      per-channel sum (ACT copy accum) / sumsq (ACT Square accum from PSUM).
  AllReduce stats -> BN affine.
  P1/P2: when all gammas>0, relu(a*z+b) = a*max(z,t)+b with t = bmu-beta/a;
      the per-channel +b term is folded into the next matmul as a bias
      (c = W@b) applied in the PSUM->z-store copy. One DVE op per chunk.
  P3: fused relu-affine -> matmul w_out (+wot@b bias) -> logits -> DRAM.
"""

import numpy as np

N_NODES = 50000
D = 256
E_TOT = 262144
NCORES = 8
ESH = E_TOT // NCORES          # 32768 edges per core
EBLK = 512                     # edges per block
NB = ESH // EBLK               # 64 blocks
EROW = ESH // 128              # 256 gather rows per partition
CH = 4096                      # edges per streamed chunk (8 blocks)
CB = CH // EBLK                # blocks per chunk
EPS = 1e-5
NL = 3


def build_nc(num_devices=NCORES, fused=True):
    import concourse.mybir as mybir
    import concourse.tile as tile
    from concourse import bacc

    f32 = mybir.dt.float32
    bf16 = mybir.dt.bfloat16
    A = mybir.ActivationFunctionType
    ALU = mybir.AluOpType
    AX = mybir.AxisListType

    groups_dev = [list(range(num_devices))]
    inv_d = 1.0 / D
    inv_e = 1.0 / E_TOT

    nc = bacc.Bacc("TRN2", target_bir_lowering=False, debug=False,
                   num_devices=num_devices)

    # ---- kernel I/O ----
    hin = nc.dram_tensor("hin", [128, EROW, D], bf16, kind="ExternalInput").ap()
    wts = [nc.dram_tensor(f"w{i}t", [D, D], bf16, kind="ExternalInput").ap()
           for i in range(NL)]
    wot = nc.dram_tensor("wot", [D, 1], bf16, kind="ExternalInput").ap()
    gam = nc.dram_tensor("gam", [NL, D], f32, kind="ExternalInput").ap()
    bet = nc.dram_tensor("bet", [NL, D], f32, kind="ExternalInput").ap()
    out = nc.dram_tensor("out", [ESH], f32, kind="ExternalOutput").ap()

    ccin = [nc.dram_tensor(f"ccin{i}", [128, 4], f32, kind="Internal").ap()
            for i in range(NL)]
    cc_space = "Shared" if num_devices > 4 else "Local"
    ccout = [nc.dram_tensor(f"ccout{i}", [128, 4], f32, kind="Internal",
                            addr_space=cc_space).ap()
             for i in range(NL)]

    with tile.TileContext(nc) as tc:
        with (
            tc.tile_pool(name="const", bufs=1) as cpool,
            tc.tile_pool(name="io", bufs=2) as iop,
            tc.tile_pool(name="work", bufs=3) as wp,
            tc.tile_pool(name="work2", bufs=2) as wp2,
            tc.tile_pool(name="stats", bufs=1) as sp,
            tc.tile_pool(name="psum", bufs=2, space="PSUM") as pp,
            tc.tile_pool(name="psum2", bufs=2, space="PSUM") as pp2,
        ):
            # ---- persistent SBUF state ----
            zst = cpool.tile([128, 2, ESH], bf16, name="zst")   # z store
            wsb = []
            for i in range(NL):
                chunks = []
                for c in range(2):
                    t = cpool.tile([128, D], bf16, name=f"w{i}c{c}")
                    nc.sync.dma_start(out=t[:], in_=wts[i][c * 128:(c + 1) * 128, :])
                    chunks.append(t)
                wsb.append(chunks)
            wot_sb = []
            for c in range(2):
                t = cpool.tile([128, 1], bf16, name=f"wo{c}")
                nc.sync.dma_start(out=t[:], in_=wot[c * 128:(c + 1) * 128, :])
                wot_sb.append(t)
            gam_sb, bet_sb = [], []
            for i in range(NL):
                g = cpool.tile([128, 2], f32, name=f"gam{i}")
                b = cpool.tile([128, 2], f32, name=f"bet{i}")
                for c in range(2):
                    nc.sync.dma_start(out=g[:, c:c + 1],
                                      in_=gam[i, c * 128:(c + 1) * 128])
                    nc.sync.dma_start(out=b[:, c:c + 1],
                                      in_=bet[i, c * 128:(c + 1) * 128])
                gam_sb.append(g)
                bet_sb.append(b)

            NP2 = NB // 2
            Sz = [[sp.tile([128, NP2], f32, name=f"Sz{i}_{c}") for c in range(2)]
                  for i in range(NL)]
            SSz = [[sp.tile([128, NP2], f32, name=f"SSz{i}_{c}") for c in range(2)]
                   for i in range(NL)]
            a_ab = [sp.tile([128, 2], f32, name=f"a{i}") for i in range(NL)]
            b_ab = [sp.tile([128, 2], f32, name=f"b{i}") for i in range(NL)]
            t_ab = [sp.tile([128, 2], f32, name=f"t{i}") for i in range(NL)]
            # +W_{li}@b_{li-1} bias for the PSUM->z-store copy (fused path)
            cbias = [sp.tile([128, 2], f32, name=f"cb{i}") for i in range(NL)]
            obias = sp.tile([1, 1], f32, name="obias")

            def produce_z2(li, pb, rhs_of, sq_on_act):
                """paired matmul z_li (2 blocks) -> z-store + stat accum.
                rhs_of(half, k) -> AP [128, 512] bf16."""
                use_bias = fused and li > 0
                for c in range(2):
                    zps = pp.tile([128, 2, EBLK], f32, name="zps", tag="zps")
                    for half in range(2):
                        for k in range(2):
                            nc.tensor.matmul(
                                out=zps[:, half, :],
                                lhsT=wsb[li][k][:, c * 128:(c + 1) * 128],
                                rhs=rhs_of(half, k),
                                start=(k == 0), stop=(k == 1))
                    zcol = zst[:, c, pb * 2 * EBLK:(pb + 1) * 2 * EBLK]
                    zpsf = zps[:].rearrange("p a e -> p (a e)")
                    if use_bias:
                        nc.scalar.activation(
                            out=zcol, in_=zpsf, func=A.Identity,
                            bias=cbias[li][:, c:c + 1],
                            accum_out=Sz[li][c][:, pb:pb + 1])
                    else:
                        nc.scalar.activation(
                            out=zcol, in_=zpsf, func=A.Copy,
                            accum_out=Sz[li][c][:, pb:pb + 1])
                    zqs = wp2.tile([128, 2 * EBLK], bf16, name="zqs", tag="zqs")
                    if sq_on_act:
                        # pre-bias PSUM squares would be wrong when use_bias;
                        # read the biased z-store columns instead
                        src = zcol if use_bias else zpsf
                        nc.scalar.activation(
                            out=zqs[:], in_=src, func=A.Square,
                            accum_out=SSz[li][c][:, pb:pb + 1])
                    else:
                        nc.vector.scalar_tensor_tensor(
                            out=zqs[:], in0=zcol, scalar=1.0, in1=zcol,
                            op0=ALU.mult, op1=ALU.mult,
                            accum_out=SSz[li][c][:, pb:pb + 1])

            # ================= Phase 0: LN + layer 0 =================
            for ci in range(ESH // CH):
                hch = iop.tile([128, CB * 4, D], bf16, name="hch", tag="hch")
                nc.sync.dma_start(
                    out=hch[:],
                    in_=hin[:, ci * (CB * 4):(ci + 1) * (CB * 4), :])
                Sln = wp.tile([128, CB * 4], f32, name="Sln", tag="Sln")
                SSln = wp.tile([128, CB * 4], f32, name="SSln", tag="SSln")
                for b in range(CB):
                    for j in range(4):
                        g = 4 * b + j
                        scr = wp.tile([128, D], bf16, name="scr", tag="scr")
                        nc.vector.tensor_scalar(
                            out=scr[:], in0=hch[:, g, :], scalar1=1.0,
                            scalar2=0.0, op0=ALU.mult, op1=ALU.add,
                            accum_out=Sln[:, g:g + 1])
                        nc.vector.scalar_tensor_tensor(
                            out=scr[:], in0=hch[:, g, :], scalar=1.0,
                            in1=hch[:, g, :], op0=ALU.mult, op1=ALU.mult,
                            accum_out=SSln[:, g:g + 1])
                # batched LN scalar math for the whole chunk [128, 32]
                mu = wp.tile([128, CB * 4], f32, name="mu", tag="mu")
                mu2 = wp.tile([128, CB * 4], f32, name="mu2", tag="mu2")
                var = wp.tile([128, CB * 4], f32, name="var", tag="var")
                inv = wp.tile([128, CB * 4], f32, name="inv", tag="inv")
                rs = wp.tile([128, CB * 4], f32, name="rs", tag="rs")
                bneg = wp.tile([128, CB * 4], f32, name="bneg", tag="bneg")
                nc.vector.tensor_scalar(
                    out=mu[:], in0=Sln[:], scalar1=inv_d, scalar2=None,
                    op0=ALU.mult)
                nc.vector.scalar_tensor_tensor(
                    out=mu2[:], in0=mu[:], scalar=1.0, in1=mu[:],
                    op0=ALU.mult, op1=ALU.mult)
                nc.vector.scalar_tensor_tensor(
                    out=var[:], in0=SSln[:], scalar=inv_d, in1=mu2[:],
                    op0=ALU.mult, op1=ALU.subtract)
                nc.vector.tensor_scalar_add(out=var[:], in0=var[:], scalar1=EPS)
                nc.vector.reciprocal(out=inv[:], in_=var[:])
                nc.scalar.sqrt(out=rs[:], in_=inv[:])
                nc.vector.scalar_tensor_tensor(
                    out=bneg[:], in0=mu[:], scalar=-1.0, in1=rs[:],
                    op0=ALU.mult, op1=ALU.mult)
                hTs = []
                for b in range(CB):
                    blk = ci * CB + b
                    hn = wp.tile([128, 4, D], bf16, name="hn", tag="hn")
                    for j in range(4):
                        g = 4 * b + j
                        nc.vector.tensor_scalar(
                            out=hn[:, j, :], in0=hch[:, g, :],
                            scalar1=rs[:, g:g + 1], scalar2=bneg[:, g:g + 1],
                            op0=ALU.mult, op1=ALU.add)
                    # xbar transpose: [128e, (g,c,p)] -> [128p, g, c, 128e]
                    hT = wp2.tile([128, 4, 2, 128], bf16, name="hT",
                                 tag=f"hT{b % 2}")
                    nc.sync.dma_start_transpose(
                        out=hT[:], in_=hn[:].rearrange("p a d -> p (a d)"))
                    hTs.append(hT)
                    if b % 2 == 1:
                        produce_z2(0, blk // 2,
                                   lambda half, k: hTs[half][:, :, k, :],
                                   sq_on_act=True)
                        hTs = []

            # ============ stats AllReduce + BN affine ============
            def finalize_stats(li):
                st4 = sp.tile([128, 4], f32, name=f"st4_{li}")
                for c in range(2):
                    nc.vector.reduce_sum(out=st4[:, c:c + 1], in_=Sz[li][c][:],
                                         axis=AX.X)
                    nc.vector.reduce_sum(out=st4[:, 2 + c:3 + c],
                                         in_=SSz[li][c][:], axis=AX.X)
                nc.sync.dma_start(out=ccin[li][:, :], in_=st4[:])
                if num_devices == 1:
                    nc.sync.dma_start(out=ccout[li][:, :], in_=ccin[li][:, :])
                else:
                    nc.gpsimd.collective_compute(
                        "AllReduce", ALU.add, replica_groups=groups_dev,
                        ins=[ccin[li][:, :]], outs=[ccout[li][:, :]])
                gst = sp.tile([128, 4], f32, name=f"gst{li}")
                nc.sync.dma_start(out=gst[:], in_=ccout[li][:, :])
                bmu = sp.tile([128, 2], f32, name=f"bmu{li}")
                bmu2 = sp.tile([128, 2], f32, name=f"bmu2{li}")
                bvar = sp.tile([128, 2], f32, name=f"bvar{li}")
                binv = sp.tile([128, 2], f32, name=f"binv{li}")
                brs = sp.tile([128, 2], f32, name=f"brs{li}")
                tt = sp.tile([128, 2], f32, name=f"tt{li}")
                nc.scalar.mul(out=bmu[:], in_=gst[:, 0:2], mul=inv_e)
                nc.scalar.square(out=bmu2[:], in_=bmu[:])
                nc.vector.scalar_tensor_tensor(
                    out=bvar[:], in0=gst[:, 2:4], scalar=inv_e, in1=bmu2[:],
                    op0=ALU.mult, op1=ALU.subtract)
                nc.vector.tensor_scalar_add(out=bvar[:], in0=bvar[:], scalar1=EPS)
                nc.vector.reciprocal(out=binv[:], in_=bvar[:])
                nc.scalar.sqrt(out=brs[:], in_=binv[:])
                nc.vector.tensor_mul(out=a_ab[li][:], in0=gam_sb[li][:], in1=brs[:])
                nc.vector.tensor_mul(out=tt[:], in0=a_ab[li][:], in1=bmu[:])
                nc.vector.tensor_sub(out=b_ab[li][:], in0=bet_sb[li][:], in1=tt[:])
                if fused:
                    # t = bmu - beta/a ;  c_{li+1} = W_{li+1} @ b ; obias = wot@b
                    ainv = sp.tile([128, 2], f32, name=f"ainv{li}")
                    boa = sp.tile([128, 2], f32, name=f"boa{li}")
                    nc.vector.reciprocal(out=ainv[:], in_=a_ab[li][:])
                    nc.vector.tensor_mul(out=boa[:], in0=bet_sb[li][:],
                                         in1=ainv[:])
                    nc.vector.tensor_sub(out=t_ab[li][:], in0=bmu[:], in1=boa[:])
                    bb = sp.tile([128, 2], bf16, name=f"bb{li}")
                    nc.scalar.copy(out=bb[:], in_=b_ab[li][:])
                    if li + 1 < NL:
                        cps = pp.tile([128, 2, EBLK], f32, name="cps",
                                      tag="zps")
                        for c in range(2):
                            for k in range(2):
                                nc.tensor.matmul(
                                    out=cps[:, 0, c:c + 1],
                                    lhsT=wsb[li + 1][k][:, c * 128:(c + 1) * 128],
                                    rhs=bb[:, k:k + 1],
                                    start=(k == 0), stop=(k == 1))
                        nc.scalar.copy(out=cbias[li + 1][:], in_=cps[:, 0, 0:2])
                    else:
                        ops = pp.tile([128, 2, EBLK], f32, name="ops",
                                      tag="zps")
                        for k in range(2):
                            nc.tensor.matmul(out=ops[0:1, 0, 0:1],
                                             lhsT=wot_sb[k][:],
                                             rhs=bb[:, k:k + 1],
                                             start=(k == 0), stop=(k == 1))
                        nc.scalar.copy(out=obias[:], in_=ops[0:1, 0, 0:1])

            finalize_stats(0)

            # ================= Phases 1..2 =================
            def relu_affine2(li, pb, c):
                """hn_c = relu(a*z + b) for a 2-block pair [128, 1024].
                Fused: a*max(z,t); the +b is folded into the next bias."""
                zcol = zst[:, c, pb * 2 * EBLK:(pb + 1) * 2 * EBLK]
                hn_c = wp2.tile([128, 2 * EBLK], bf16, name=f"rhc{c}",
                               tag=f"rhc{c}")
                if fused:
                    nc.vector.tensor_scalar(
                        out=hn_c[:], in0=zcol,
                        scalar1=t_ab[li - 1][:, c:c + 1],
                        scalar2=a_ab[li - 1][:, c:c + 1],
                        op0=ALU.max, op1=ALU.mult)
                else:
                    t = wp2.tile([128, 2 * EBLK], bf16, name=f"aff{c}",
                                tag=f"aff{c}")
                    nc.vector.tensor_scalar(
                        out=t[:], in0=zcol, scalar1=a_ab[li - 1][:, c:c + 1],
                        scalar2=b_ab[li - 1][:, c:c + 1], op0=ALU.mult,
                        op1=ALU.add)
                    nc.vector.tensor_scalar(
                        out=hn_c[:], in0=t[:], scalar1=0.0, scalar2=None,
                        op0=ALU.max)
                return hn_c

            for li in range(1, NL):
                for pb in range(NB // 2):
                    hns = [relu_affine2(li, pb, c) for c in range(2)]
                    produce_z2(li, pb,
                               lambda half, k: hns[k][:, half * EBLK:
                                                      (half + 1) * EBLK],
                               sq_on_act=False)
                finalize_stats(li)

            # ================= Phase 3: final projection =================
            for pb in range(NB // 2):
                hns = [relu_affine2(NL, pb, c) for c in range(2)]
                lps = pp2.tile([1, 2, EBLK], f32, name="lps", tag="lps")
                for half in range(2):
                    for c in range(2):
                        nc.tensor.matmul(
                            out=lps[:, half, :], lhsT=wot_sb[c][:],
                            rhs=hns[c][:, half * EBLK:(half + 1) * EBLK],
                            start=(c == 0), stop=(c == 1))
                lsb = wp2.tile([1, 2 * EBLK], f32, name="lsb", tag="lsb")
                lpsf = lps[:].rearrange("p a e -> p (a e)")
                if fused:
                    nc.scalar.activation(out=lsb[:], in_=lpsf, func=A.Identity,
                                         bias=obias[:, 0:1])
                else:
                    nc.scalar.copy(out=lsb[:], in_=lpsf)
                nc.sync.dma_start(out=out[pb * 2 * EBLK:(pb + 1) * 2 * EBLK],
                                  in_=lsb[:])

    nc.compile()
    return nc


_NC = None
_NC_KEY = None


def _to_bf16(a):
    import ml_dtypes
    return np.asarray(a, dtype=np.float32).astype(ml_dtypes.bfloat16)


def kernel(**inputs):
    global _NC, _NC_KEY

    x = np.asarray(inputs["x"], dtype=np.float32)
    ei = np.asarray(inputs["jg_edge_index"]).astype(np.int64)
    ln_w = np.asarray(inputs["ln_w"], dtype=np.float32)
    Ws = np.asarray(inputs["Ws"], dtype=np.float32)
    gammas = np.asarray(inputs["gammas"], dtype=np.float32)
    betas = np.asarray(inputs["betas"], dtype=np.float32)
    W_out = np.asarray(inputs["W_out"], dtype=np.float32)

    # fold ln_w into layer-0 weight; lhsT layout = W.T ([in,out])
    W0f = Ws[0] * ln_w[None, :]
    wts = [_to_bf16(W0f.T), _to_bf16(Ws[1].T), _to_bf16(Ws[2].T)]
    wot = _to_bf16(W_out.reshape(1, D).T)

    # relu(a*z+b) = a*max(z, t)+b identity requires a = gamma*rsqrt(var) > 0
    fused = bool((gammas > 0).all())

    # host-side gather of the edge features (data-parallel sharding of
    # "gathered edge features" per the sharding strategy)
    h = x[ei[0], :] + x[ei[1], :]                    # [E, 256] fp32
    h_bf = _to_bf16(h)

    if _NC is None or _NC_KEY != fused:
        _NC = build_nc(fused=fused)
        _NC_KEY = fused

    in_maps = []
    for c in range(NCORES):
        hc = h_bf[c * ESH:(c + 1) * ESH]
        # edge e -> (partition e%128, row e//128) device layout
        hdev = np.ascontiguousarray(
            hc.reshape(EROW, 128, D).transpose(1, 0, 2))
        in_maps.append({
            "hin": hdev,
            "w0t": wts[0], "w1t": wts[1], "w2t": wts[2],
            "wot": wot,
            "gam": gammas,
            "bet": betas,
        })

    global _last_in_maps
    _last_in_maps = in_maps

    from concourse import bass_utils
    res = bass_utils.run_bass_kernel_spmd(_NC, in_maps, core_ids=list(range(NCORES)))
    return np.concatenate([np.asarray(res.results[c]["out"], dtype=np.float32)
                           for c in range(NCORES)], axis=0)


_last_in_maps = None


# revision 27
# speedup vs baseline: 1.5453x; 1.1427x over previous
"""Trainium2 Bass kernel for nn_JointPairHead: edge gather + LN + 3x(Linear->BN->ReLU) -> logits.

Sharding (per the data-parallel hint): shard edge_index and gathered edge
features across 8 cores; params replicated. The edge-feature gather
h = x[src] + x[dst] happens host-side (numpy fancy indexing) and each core
receives its contiguous [E/8, 256] bf16 slab pre-arranged in the on-device
edge-major tile layout. BN batch stats cross-core via AllReduce of per-shard
sum/sumsq.

Device dataflow (per core, E_shard = 32768 edges, 64 blocks of 512,
all-bf16 compute with fp32 stat accumulation, SBUF-resident activations):
  P0: stream h blocks -> LN stats (sum via tensor_scalar accum, sumsq via
      scalar_tensor_tensor accum; scalar math batched per 8-block chunk)
      -> normalize via tensor_scalar -> xbar DMA transpose to feature-major
      -> matmul z0 = W0f @ hnT (ln_w folded into W0) -> z0 into z-store with
      per-channel sum (ACT copy accum) / sumsq (ACT Square accum from PSUM).
  AllReduce stats -> BN affine.
  P1/P2: when all gammas>0, relu(a*z+b) = a*max(z,t)+b with t = bmu-beta/a;
      the per-channel +b term is folded into the next matmul as a bias
      (c = W@b) applied in the PSUM->z-store copy. One DVE op per chunk.
  P3: fused relu-affine -> matmul w_out (+wot@b bias) -> logits -> DRAM.
"""

import numpy as np

N_NODES = 50000
D = 256
E_TOT = 262144
NCORES = 8
ESH = E_TOT // NCORES          # 32768 edges per core
EBLK = 512                     # edges per block
NB = ESH // EBLK               # 64 blocks
EROW = ESH // 128              # 256 gather rows per partition
CH = 2048                      # edges per streamed chunk (4 blocks)
CB = CH // EBLK                # blocks per chunk
EPS = 1e-5
NL = 3


def build_nc(num_devices=NCORES, fused=True):
    import concourse.mybir as mybir
    import concourse.tile as tile
    from concourse import bacc

    f32 = mybir.dt.float32
    bf16 = mybir.dt.bfloat16
    A = mybir.ActivationFunctionType
    ALU = mybir.AluOpType
    AX = mybir.AxisListType

    groups_dev = [list(range(num_devices))]
    inv_d = 1.0 / D
    inv_e = 1.0 / E_TOT

    nc = bacc.Bacc("TRN2", target_bir_lowering=False, debug=False,
                   num_devices=num_devices)

    # ---- kernel I/O ----
    hin = nc.dram_tensor("hin", [128, EROW, D], bf16, kind="ExternalInput").ap()
    wts = [nc.dram_tensor(f"w{i}t", [D, D], bf16, kind="ExternalInput").ap()
           for i in range(NL)]
    wot = nc.dram_tensor("wot", [D, 1], bf16, kind="ExternalInput").ap()
    gam = nc.dram_tensor("gam", [NL, D], f32, kind="ExternalInput").ap()
    bet = nc.dram_tensor("bet", [NL, D], f32, kind="ExternalInput").ap()
    out = nc.dram_tensor("out", [ESH], f32, kind="ExternalOutput").ap()
    b2o = nc.dram_tensor("b2o", [128, 2], f32, kind="ExternalOutput").ap()

    ccin = [nc.dram_tensor(f"ccin{i}", [128, 4], f32, kind="Internal").ap()
            for i in range(NL)]
    cc_space = "Shared" if num_devices > 4 else "Local"
    ccout = [nc.dram_tensor(f"ccout{i}", [128, 4], f32, kind="Internal",
                            addr_space=cc_space).ap()
             for i in range(NL)]

    with tile.TileContext(nc) as tc:
        with (
            tc.tile_pool(name="const", bufs=1) as cpool,
            tc.tile_pool(name="io", bufs=3) as iop,
            tc.tile_pool(name="work", bufs=3) as wp,
            tc.tile_pool(name="work2", bufs=2) as wp2,
            tc.tile_pool(name="work3", bufs=3) as wp3,
            tc.tile_pool(name="stats", bufs=1) as sp,
            tc.tile_pool(name="psum", bufs=4, space="PSUM") as pp,
        ):
            # ---- persistent SBUF state ----
            zst = cpool.tile([128, 2, ESH], bf16, name="zst")   # z store
            wsb = []
            for i in range(NL):
                chunks = []
                for c in range(2):
                    t = cpool.tile([128, D], bf16, name=f"w{i}c{c}")
                    nc.sync.dma_start(out=t[:], in_=wts[i][c * 128:(c + 1) * 128, :])
                    chunks.append(t)
                wsb.append(chunks)
            wot_sb = []
            for c in range(2):
                t = cpool.tile([128, 1], bf16, name=f"wo{c}")
                nc.sync.dma_start(out=t[:], in_=wot[c * 128:(c + 1) * 128, :])
                wot_sb.append(t)
            gam_sb, bet_sb = [], []
            for i in range(NL):
                g = cpool.tile([128, 2], f32, name=f"gam{i}")
                b = cpool.tile([128, 2], f32, name=f"bet{i}")
                for c in range(2):
                    nc.sync.dma_start(out=g[:, c:c + 1],
                                      in_=gam[i, c * 128:(c + 1) * 128])
                    nc.sync.dma_start(out=b[:, c:c + 1],
                                      in_=bet[i, c * 128:(c + 1) * 128])
                gam_sb.append(g)
                bet_sb.append(b)

            NP2 = NB // 2
            Sz = [[sp.tile([128, NP2], f32, name=f"Sz{i}_{c}") for c in range(2)]
                  for i in range(NL)]
            SSz = [[sp.tile([128, NP2], f32, name=f"SSz{i}_{c}") for c in range(2)]
                   for i in range(NL)]
            a_ab = [sp.tile([128, 2], f32, name=f"a{i}") for i in range(NL)]
            b_ab = [sp.tile([128, 2], f32, name=f"b{i}") for i in range(NL)]
            t_ab = [sp.tile([128, 2], f32, name=f"t{i}") for i in range(NL)]
            # +W_{li}@b_{li-1} bias for the PSUM->z-store copy (fused path)
            cbias = [sp.tile([128, 2], f32, name=f"cb{i}") for i in range(NL)]

            def produce_z2(li, pb, rhs_of, sq_on_act):
                """paired matmul z_li (2 blocks) -> z-store + stat accum.
                rhs_of(half, k) -> AP [128, 512] bf16."""
                use_bias = fused and li > 0
                for c in range(2):
                    zps = pp.tile([128, 2, EBLK], f32, name="zps", tag="zps")
                    for half in range(2):
                        for k in range(2):
                            nc.tensor.matmul(
                                out=zps[:, half, :],
                                lhsT=wsb[li][k][:, c * 128:(c + 1) * 128],
                                rhs=rhs_of(half, k),
                                start=(k == 0), stop=(k == 1))
                    zcol = zst[:, c, pb * 2 * EBLK:(pb + 1) * 2 * EBLK]
                    zpsf = zps[:].rearrange("p a e -> p (a e)")
                    if use_bias:
                        nc.scalar.activation(
                            out=zcol, in_=zpsf, func=A.Identity,
                            bias=cbias[li][:, c:c + 1],
                            accum_out=Sz[li][c][:, pb:pb + 1])
                    else:
                        nc.scalar.activation(
                            out=zcol, in_=zpsf, func=A.Copy,
                            accum_out=Sz[li][c][:, pb:pb + 1])
                    zqs = wp2.tile([128, 2 * EBLK], bf16, name="zqs", tag="zqs")
                    if sq_on_act:
                        # pre-bias PSUM squares would be wrong when use_bias;
                        # read the biased z-store columns instead
                        src = zcol if use_bias else zpsf
                        nc.scalar.activation(
                            out=zqs[:], in_=src, func=A.Square,
                            accum_out=SSz[li][c][:, pb:pb + 1])
                    else:
                        nc.vector.tensor_tensor(out=zqs[:], in0=zcol, in1=zcol,
                                                op=ALU.mult)
                        nc.vector.tensor_scalar(
                            out=zqs[:], in0=zqs[:], scalar1=1.0, scalar2=0.0,
                            op0=ALU.mult, op1=ALU.add,
                            accum_out=SSz[li][c][:, pb:pb + 1])

            # ================= Phase 0: LN + layer 0 =================
            # software-pipelined: stats+LN math of chunk n run before the
            # normalize/matmul pass of chunk n-1 so the ACT sqrt is not
            # head-of-line blocked behind the previous chunk's zsb/zsq.
            def p0_pass_a(ci):
                hch = iop.tile([128, CB * 4, D], bf16, name="hch", tag="hch")
                nc.sync.dma_start(
                    out=hch[:],
                    in_=hin[:, ci * (CB * 4):(ci + 1) * (CB * 4), :])
                Sln = wp.tile([128, CB * 4], f32, name="Sln", tag="Sln")
                SSln = wp.tile([128, CB * 4], f32, name="SSln", tag="SSln")
                for b in range(CB):
                    for j in range(4):
                        g = 4 * b + j
                        scr = wp.tile([128, D], bf16, name="scr", tag="scr")
                        nc.vector.tensor_scalar(
                            out=scr[:], in0=hch[:, g, :], scalar1=1.0,
                            scalar2=0.0, op0=ALU.mult, op1=ALU.add,
                            accum_out=Sln[:, g:g + 1])
                        nc.vector.scalar_tensor_tensor(
                            out=scr[:], in0=hch[:, g, :], scalar=1.0,
                            in1=hch[:, g, :], op0=ALU.mult, op1=ALU.mult,
                            accum_out=SSln[:, g:g + 1])
                # batched LN scalar math for the whole chunk
                mu = wp.tile([128, CB * 4], f32, name="mu", tag="mu")
                mu2 = wp.tile([128, CB * 4], f32, name="mu2", tag="mu2")
                var = wp.tile([128, CB * 4], f32, name="var", tag="var")
                inv = wp.tile([128, CB * 4], f32, name="inv", tag="inv")
                rs = wp.tile([128, CB * 4], f32, name="rs", tag="rs")
                bneg = wp.tile([128, CB * 4], f32, name="bneg", tag="bneg")
                nc.vector.tensor_scalar(
                    out=mu[:], in0=Sln[:], scalar1=inv_d, scalar2=None,
                    op0=ALU.mult)
                nc.vector.scalar_tensor_tensor(
                    out=mu2[:], in0=mu[:], scalar=1.0, in1=mu[:],
                    op0=ALU.mult, op1=ALU.mult)
                nc.vector.scalar_tensor_tensor(
                    out=var[:], in0=SSln[:], scalar=inv_d, in1=mu2[:],
                    op0=ALU.mult, op1=ALU.subtract)
                nc.vector.tensor_scalar_add(out=var[:], in0=var[:],
                                            scalar1=EPS)
                nc.vector.reciprocal(out=inv[:], in_=var[:])
                nc.scalar.sqrt(out=rs[:], in_=inv[:])
                nc.vector.scalar_tensor_tensor(
                    out=bneg[:], in0=mu[:], scalar=-1.0, in1=rs[:],
                    op0=ALU.mult, op1=ALU.mult)
                return hch, rs, bneg

            def p0_pass_b(ci, hch, rs, bneg):
                hTs = []
                for b in range(CB):
                    blk = ci * CB + b
                    hn = wp.tile([128, 4, D], bf16, name="hn", tag="hn")
                    for j in range(4):
                        g = 4 * b + j
                        nc.vector.tensor_scalar(
                            out=hn[:, j, :], in0=hch[:, g, :],
                            scalar1=rs[:, g:g + 1], scalar2=bneg[:, g:g + 1],
                            op0=ALU.mult, op1=ALU.add)
                    # xbar transpose: [128e, (g,c,p)] -> [128p, g, c, 128e]
                    hT = wp2.tile([128, 4, 2, 128], bf16, name="hT",
                                  tag=f"hT{b % 2}")
                    nc.sync.dma_start_transpose(
                        out=hT[:], in_=hn[:].rearrange("p a d -> p (a d)"))
                    hTs.append(hT)
                    if b % 2 == 1:
                        produce_z2(0, blk // 2,
                                   lambda half, k: hTs[half][:, :, k, :],
                                   sq_on_act=True)
                        hTs = []

            prev = None
            for ci in range(ESH // CH):
                cur = p0_pass_a(ci)
                if prev is not None:
                    p0_pass_b(prev[0], *prev[1])
                prev = (ci, cur)
            p0_pass_b(prev[0], *prev[1])

            # ============ stats AllReduce + BN affine ============
            def finalize_stats(li):
                st4 = sp.tile([128, 4], f32, name=f"st4_{li}")
                for c in range(2):
                    nc.vector.reduce_sum(out=st4[:, c:c + 1], in_=Sz[li][c][:],
                                         axis=AX.X)
                    nc.vector.reduce_sum(out=st4[:, 2 + c:3 + c],
                                         in_=SSz[li][c][:], axis=AX.X)
                nc.sync.dma_start(out=ccin[li][:, :], in_=st4[:])
                if num_devices == 1:
                    nc.sync.dma_start(out=ccout[li][:, :], in_=ccin[li][:, :])
                else:
                    nc.gpsimd.collective_compute(
                        "AllReduce", ALU.add, replica_groups=groups_dev,
                        ins=[ccin[li][:, :]], outs=[ccout[li][:, :]])
                gst = sp.tile([128, 4], f32, name=f"gst{li}")
                nc.sync.dma_start(out=gst[:], in_=ccout[li][:, :])
                bmu = sp.tile([128, 2], f32, name=f"bmu{li}")
                bmu2 = sp.tile([128, 2], f32, name=f"bmu2{li}")
                bvar = sp.tile([128, 2], f32, name=f"bvar{li}")
                binv = sp.tile([128, 2], f32, name=f"binv{li}")
                brs = sp.tile([128, 2], f32, name=f"brs{li}")
                tt = sp.tile([128, 2], f32, name=f"tt{li}")
                nc.scalar.mul(out=bmu[:], in_=gst[:, 0:2], mul=inv_e)
                nc.scalar.square(out=bmu2[:], in_=bmu[:])
                nc.vector.scalar_tensor_tensor(
                    out=bvar[:], in0=gst[:, 2:4], scalar=inv_e, in1=bmu2[:],
                    op0=ALU.mult, op1=ALU.subtract)
                nc.vector.tensor_scalar_add(out=bvar[:], in0=bvar[:], scalar1=EPS)
                nc.vector.reciprocal(out=binv[:], in_=bvar[:])
                nc.scalar.sqrt(out=brs[:], in_=binv[:])
                nc.vector.tensor_mul(out=a_ab[li][:], in0=gam_sb[li][:], in1=brs[:])
                nc.vector.tensor_mul(out=tt[:], in0=a_ab[li][:], in1=bmu[:])
                nc.vector.tensor_sub(out=b_ab[li][:], in0=bet_sb[li][:], in1=tt[:])
                if fused:
                    # t = bmu - beta/a ;  c_{li+1} = W_{li+1} @ b ; obias = wot@b
                    ainv = sp.tile([128, 2], f32, name=f"ainv{li}")
                    boa = sp.tile([128, 2], f32, name=f"boa{li}")
                    nc.vector.reciprocal(out=ainv[:], in_=a_ab[li][:])
                    nc.vector.tensor_mul(out=boa[:], in0=bet_sb[li][:],
                                         in1=ainv[:])
                    nc.vector.tensor_sub(out=t_ab[li][:], in0=bmu[:], in1=boa[:])
                    bb = sp.tile([128, 2], bf16, name=f"bb{li}")
                    nc.scalar.copy(out=bb[:], in_=b_ab[li][:])
                    if li + 1 < NL:  # wot bias handled host-free (P3 unfused)
                        cps = pp.tile([128, 2, EBLK], f32, name="cps",
                                      tag="zps")
                        for c in range(2):
                            for k in range(2):
                                nc.tensor.matmul(
                                    out=cps[:, 0, c:c + 1],
                                    lhsT=wsb[li + 1][k][:, c * 128:(c + 1) * 128],
                                    rhs=bb[:, k:k + 1],
                                    start=(k == 0), stop=(k == 1))
                        nc.scalar.copy(out=cbias[li + 1][:], in_=cps[:, 0, 0:2])


            finalize_stats(0)

            # ================= Phases 1..2 =================
            def relu_affine2(li, pb, c, force_unfused=False):
                """hn_c = relu(a*z + b) for a 2-block pair [128, 1024].
                Fused: a*max(z,t); the +b is folded into the next bias."""
                zcol = zst[:, c, pb * 2 * EBLK:(pb + 1) * 2 * EBLK]
                hn_c = wp3.tile([128, 2 * EBLK], bf16, name=f"rhc{c}",
                               tag=f"rhc{c}")
                if fused and not force_unfused:
                    nc.vector.tensor_scalar(
                        out=hn_c[:], in0=zcol,
                        scalar1=t_ab[li - 1][:, c:c + 1],
                        scalar2=a_ab[li - 1][:, c:c + 1],
                        op0=ALU.max, op1=ALU.mult)
                else:
                    t = wp2.tile([128, 2 * EBLK], bf16, name=f"aff{c}",
                                tag=f"aff{c}")
                    nc.vector.tensor_scalar(
                        out=t[:], in0=zcol, scalar1=a_ab[li - 1][:, c:c + 1],
                        scalar2=b_ab[li - 1][:, c:c + 1], op0=ALU.mult,
                        op1=ALU.add)
                    nc.vector.tensor_scalar(
                        out=hn_c[:], in0=t[:], scalar1=0.0, scalar2=None,
                        op0=ALU.max)
                return hn_c

            for li in range(1, NL):
                for pb in range(NB // 2):
                    hns = [relu_affine2(li, pb, c) for c in range(2)]
                    produce_z2(li, pb,
                               lambda half, k: hns[k][:, half * EBLK:
                                                      (half + 1) * EBLK],
                               sq_on_act=False)
                finalize_stats(li)

            nc.sync.dma_start(out=b2o[:, :], in_=b_ab[NL - 1][:])

            # ================= Phase 3: final projection =================
            for pb in range(NB // 2):
                hns = [relu_affine2(NL, pb, c) for c in range(2)]
                lps = pp.tile([128, 2, EBLK], f32, name="lps", tag="zps")
                for half in range(2):
                    for c in range(2):
                        nc.tensor.matmul(
                            out=lps[0:1, half, :], lhsT=wot_sb[c][:],
                            rhs=hns[c][:, half * EBLK:(half + 1) * EBLK],
                            start=(c == 0), stop=(c == 1))
                lpsf = lps[0:1, :, :].rearrange("p a e -> p (a e)")
                lsb = wp2.tile([1, 2 * EBLK], f32, name="lsb", tag="lsb")
                nc.scalar.copy(out=lsb[:], in_=lpsf)
                nc.sync.dma_start(out=out[pb * 2 * EBLK:(pb + 1) * 2 * EBLK],
                                  in_=lsb[:])

    nc.compile()
    return nc


_NC = None
_NC_KEY = None


def _to_bf16(a):
    import ml_dtypes
    return np.asarray(a, dtype=np.float32).astype(ml_dtypes.bfloat16)


def kernel(**inputs):
    global _NC, _NC_KEY

    x = np.asarray(inputs["x"], dtype=np.float32)
    ei = np.asarray(inputs["jg_edge_index"]).astype(np.int64)
    ln_w = np.asarray(inputs["ln_w"], dtype=np.float32)
    Ws = np.asarray(inputs["Ws"], dtype=np.float32)
    gammas = np.asarray(inputs["gammas"], dtype=np.float32)
    betas = np.asarray(inputs["betas"], dtype=np.float32)
    W_out = np.asarray(inputs["W_out"], dtype=np.float32)

    # fold ln_w into layer-0 weight; lhsT layout = W.T ([in,out])
    W0f = Ws[0] * ln_w[None, :]
    wts = [_to_bf16(W0f.T), _to_bf16(Ws[1].T), _to_bf16(Ws[2].T)]
    wot = _to_bf16(W_out.reshape(1, D).T)

    # relu(a*z+b) = a*max(z, t)+b identity requires a = gamma*rsqrt(var) > 0
    fused = bool((gammas > 0).all())

    # host-side gather of the edge features (data-parallel sharding of
    # "gathered edge features" per the sharding strategy)
    h = x[ei[0], :] + x[ei[1], :]                    # [E, 256] fp32
    h_bf = _to_bf16(h)

    if _NC is None or _NC_KEY != fused:
        _NC = build_nc(fused=fused)
        _NC_KEY = fused

    in_maps = []
    for c in range(NCORES):
        hc = h_bf[c * ESH:(c + 1) * ESH]
        # edge e -> (partition e%128, row e//128) device layout
        hdev = np.ascontiguousarray(
            hc.reshape(EROW, 128, D).transpose(1, 0, 2))
        in_maps.append({
            "hin": hdev,
            "w0t": wts[0], "w1t": wts[1], "w2t": wts[2],
            "wot": wot,
            "gam": gammas,
            "bet": betas,
        })

    global _last_in_maps
    _last_in_maps = in_maps

    from concourse import bass_utils
    res = bass_utils.run_bass_kernel_spmd(_NC, in_maps, core_ids=list(range(NCORES)))
    full = np.concatenate([np.asarray(res.results[c]["out"], dtype=np.float32)
                           for c in range(NCORES)], axis=0)
    if fused:
        # P3 computes wot^T (a*max(z,t)); add the folded wot^T b term here
        b2o = np.asarray(res.results[0]["b2o"], dtype=np.float32)
        b2 = b2o.T.reshape(-1)                       # channel j = c*128+p
        full += float(wot.astype(np.float32)[:, 0] @ b2)
    return full


_last_in_maps = None
